# revision 25
# baseline (speedup 1.0000x reference)
"""Trainium2 Bass kernel for nn_CaptionDecoder (attention LSTM caption decoder).

Strategy (8 NeuronCores):
  Phase A: data-parallel over batch (8 batches/core) for the sequential
           attention+2-layer-LSTM recurrence. Produces top-layer hidden
           states hb for all 20 steps (tiny: [512, 160] bf16 per core).
  Host:    gathers hb from the 8 cores (1.3 MB total), reassembles.
  Phase B: vocab-parallel logits projection: every core gets the full
           hb and a 3750-column slice of W_out; computes the logits
           TRANSPOSED ([3750, 1280]) with W_out as the matmul stationary
           operand (full 128-wide PE); the host concatenates along vocab
           and adds b_out.

Phase A layout tricks:
  - The two LSTM gate blocks [8, 2048] are computed with PE column-tiling:
    the four gate types (f, i, C~, o) each map to one 32-column group of
    the systolic array, so their weight streams execute concurrently
    (4x fewer serial N=512 passes).
  - dec_proj and the attention context are produced directly in
    TRANSPOSED form ([a/e, batch]) with the weights/encoder slices as
    stationary operands, eliminating 8 PE transposes per step.
  - Softmax plumbing: exp writes a p-major row so the alpha scatter into
    block-column form is 2 DMAs (was 16); exp-sums come back as a psum
    row whose reciprocal feeds a per-batch-scaled selector matrix that
    both reduces the context partials and applies 1/sum.
  - The four gate activations run as ONE 104-partition ACT instruction
    (groups live at partition bases 0/32/64/96) with a per-partition
    scale vector implementing the sigmoid-as-tanh trick.

Precision: bf16 matmuls with fp32 PSUM accumulation; fp32 state and
pointwise math. Sigmoid is computed as 0.5*(1+tanh(x/2)); the 0.5
factors are folded into pre-scaled weights by keeping doubled states
H2=2h, C2=2c on device.
"""

import numpy as np
import ml_dtypes

import concourse.bass as bass
import concourse.bacc as bacc
import concourse.mybir as mybir
import concourse.tile as tile
from concourse.bass import ts
from concourse.bass_utils import run_bass_kernel_spmd
from concourse.masks import make_identity

F32 = mybir.dt.float32
BF16 = mybir.dt.bfloat16
AF = mybir.ActivationFunctionType
ALU = mybir.AluOpType

B, TC, P, E, H, A, V = 64, 21, 196, 512, 512, 512, 30000
T = TC - 1            # 20 decode steps
NC = 8                # cores
BL = B // NC          # 8 batches per core
PPAD = 256            # padded attention positions per batch
NF = BL * PPAD // 128  # 16 position chunks for context matmul
BP = BL * P           # 1568 (b, p) columns per core
G4 = 4 * H            # 2048 stacked gates f,i,C,o
VSL = V // NC         # 3750 vocab columns per core
BT = B * T            # 1280 output rows


def _bf16(x):
    return np.ascontiguousarray(np.asarray(x), dtype=None).astype(ml_dtypes.bfloat16)


def _gorder(w):
    """Reorder stacked gate blocks (f,i,C,o) -> (i,f,C,o) on the last axis."""
    w = np.asarray(w)
    blocks = w.reshape(*w.shape[:-1], 4, H)
    return blocks[..., [1, 0, 2, 3], :].reshape(w.shape)


def _sub(ap, dims, extra_offset=0):
    """Custom free-dim access pattern on an AP, keeping its partition dim."""
    return bass.AP(ap.tensor, ap.offset + extra_offset,
                   [list(ap.ap[0])] + [list(d) for d in dims])


def _pbcast(ap, dims, nparts=128, extra_offset=0):
    """Partition-broadcast (stride 0) custom AP."""
    return bass.AP(ap.tensor, ap.offset + extra_offset,
                   [[0, nparts]] + [list(d) for d in dims])


# --------------------------------------------------------------------------
# Phase A module: the recurrence
# --------------------------------------------------------------------------

def build_phase_a(n_steps=T, num_devices=NC):
    nc = bacc.Bacc("TRN2", num_devices=num_devices, debug=False)

    def din(name, shape, dt=BF16):
        return nc.dram_tensor(name, shape, dt, kind="ExternalInput").ap()

    encT = din("encT", [4, 128, BP])          # encoder_out^T  [e-chk][e][(b,p)]
    encflat = din("encflat", [NF, 128, E])    # [(b,ppad) chk][row][e], 0-padded
    featT = din("featT", [4, 128, BL])
    wih2 = din("wih2", [4, 128, H])           # 2*W_ih
    wic2 = din("wic2", [4, 128, H])           # 2*W_ic
    bih2 = din("bih2", [1, H])
    bic2 = din("bic2", [1, H])
    wenc = din("wenc", [4, 128, A])
    biasad = din("biasad", [1, A])            # b_enc + b_dec
    wdech = din("wdech", [4, 128, A])         # 0.5*W_dec
    vcol = din("vcol", [4, 128, 1])
    weT = din("weT", [4, 128, T * BL])        # embeds^T, col = t*8+b
    wg0x = din("wg0x", [4, 128, G4])
    bg0 = din("bg0", [1, G4])
    wg0c = din("wg0c", [4, 128, G4])
    wg0h = din("wg0h", [4, 128, G4])          # 0.5*
    wg1a = din("wg1a", [4, 128, G4])          # 0.5*
    wg1b = din("wg1b", [4, 128, G4])          # 0.5*
    bg1 = din("bg1", [1, G4])
    diag01 = din("diag01", [104, BL])         # 1 at (32g+b, b)

    hballT = nc.dram_tensor("hballT", [4, 128, T * BL], BF16,
                            kind="ExternalOutput").ap()
    # staging for the hoisted word-embedding part of the gate0 pre-activation
    wepart = nc.dram_tensor("wepart", [T * BL, G4], BF16).ap()

    with tile.TileContext(nc) as tc:
        with (
            tc.tile_pool(name="persist", bufs=1) as pp,
            tc.tile_pool(name="psG", bufs=2, space="PSUM") as psG,
            tc.tile_pool(name="psS", bufs=1, space="PSUM") as psS,
            tc.tile_pool(name="psT", bufs=1, space="PSUM") as psT,
        ):
            def dma3(dst, src, n=4):  # dram [n,128,X] -> sbuf [128,n,X]
                for k in range(n):
                    nc.sync.dma_start(out=dst[:, k], in_=src[k])

            # ---- persistent weights / constants --------------------------
            sb_wdech = pp.tile([128, 4, A], BF16, tag="wdech")
            dma3(sb_wdech, wdech)
            sb_vcol = pp.tile([128, 4, 1], BF16, tag="vcol")
            dma3(sb_vcol, vcol)
            sb_biasad = pp.tile([1, A], BF16, tag="biasad")
            nc.sync.dma_start(out=sb_biasad[:], in_=biasad)
            sb_wg0c = pp.tile([128, 4, G4], BF16, tag="wg0c")
            dma3(sb_wg0c, wg0c)
            sb_wg0h = pp.tile([128, 4, G4], BF16, tag="wg0h")
            dma3(sb_wg0h, wg0h)
            sb_wg1a = pp.tile([128, 4, G4], BF16, tag="wg1a")
            dma3(sb_wg1a, wg1a)
            sb_wg1b = pp.tile([128, 4, G4], BF16, tag="wg1b")
            dma3(sb_wg1b, wg1b)
            sb_bg1 = pp.tile([1, G4], BF16, tag="bg1")
            nc.sync.dma_start(out=sb_bg1[:], in_=bg1)
            sb_encflat = pp.tile([128, NF, E], BF16, tag="encflat")
            dma3(sb_encflat, encflat, n=NF)
            sb_diag = pp.tile([104, BL], BF16, tag="diag01")
            nc.sync.dma_start(out=sb_diag[:], in_=diag01)

            i8f = pp.tile([8, 8], F32, tag="i8f")
            make_identity(nc, i8f[:])
            i8b = pp.tile([8, 8], BF16, tag="i8b")
            make_identity(nc, i8b[:])
            i1b = pp.tile([1, 1], BF16, tag="i1b")
            nc.vector.memset(i1b[:], 1.0)
            i8pad = pp.tile([8, 32], BF16, tag="i8pad")
            nc.vector.memset(i8pad[:], 0.0)
            make_identity(nc, i8pad[:, 0:8], nomemset=True)
            onespad = pp.tile([1, 32], BF16, tag="onespad")
            nc.vector.memset(onespad[:], 0.0)
            nc.vector.memset(onespad[:, 0:8], 1.0)
            ones_1x8 = pp.tile([1, 8], BF16, tag="o18")
            nc.vector.memset(ones_1x8[:], 1.0)
            ones_row = pp.tile([1, BP // 4], BF16, tag="orow")
            nc.vector.memset(ones_row[:], 1.0)
            ones_1x128 = pp.tile([1, 128], BF16, tag="o1128")
            nc.vector.memset(ones_1x128[:], 1.0)
            ones_col = pp.tile([128, 1], BF16, tag="ocol")
            nc.vector.memset(ones_col[:], 1.0)
            ones_1x104 = pp.tile([1, 104], F32, tag="o1104")
            nc.vector.memset(ones_1x104[:], 1.0)
            scvec2 = pp.tile([40, 1], F32, tag="scvec2")
            nc.vector.memset(scvec2[0:32], 1.0)
            nc.vector.memset(scvec2[32:40], 0.5)

            # state
            C2a = pp.tile([40, H], F32, tag="C2a")   # state in rows 32-39
            C2b = pp.tile([40, H], F32, tag="C2b")
            H2aT = pp.tile([128, 4, 8], BF16, tag="H2aT")
            H2bT = pp.tile([128, 4, 8], BF16, tag="H2bT")
            hball_sb = pp.tile([128, 4, T * BL], BF16, tag="hball")
            nc.vector.memset(hball_sb[:], 0.0)

            # alpha block-column tile: column 17*b of chunk-slice 2b holds
            # alpha[0:128] for batch b; column 17*b+8 holds alpha[128:196].
            aB = pp.tile([128, 128], BF16, tag="aB")
            nc.vector.memset(aB[:], 0.0)

            encproj = pp.tile([128, 4, BP], BF16, tag="encproj")

            # ---- one-time section (own pool, freed before the loop) ------
            sp = tc.alloc_tile_pool(name="stream", bufs=1)
            sb_featT = sp.tile([128, 4, BL], BF16, tag="featT")
            dma3(sb_featT, featT)
            sb_wih2 = sp.tile([128, 4, H], BF16, tag="wih2")
            dma3(sb_wih2, wih2)
            sb_wic2 = sp.tile([128, 4, H], BF16, tag="wic2")
            dma3(sb_wic2, wic2)
            sb_bih2 = sp.tile([1, H], BF16, tag="bih2")
            nc.sync.dma_start(out=sb_bih2[:], in_=bih2)
            sb_bic2 = sp.tile([1, H], BF16, tag="bic2")
            nc.sync.dma_start(out=sb_bic2[:], in_=bic2)

            h0ps = psS.tile([8, H], F32, tag="onetime")
            for k in range(4):
                nc.tensor.matmul(h0ps[:], sb_featT[:, k], sb_wih2[:, k],
                                 start=(k == 0), stop=False)
            nc.tensor.matmul(h0ps[:], ones_1x8[:], sb_bih2[:],
                             start=False, stop=True)
            h0sb = sp.tile([8, H], F32, tag="h0sb")
            nc.vector.tensor_copy(h0sb[:], h0ps[:])
            smallF = psT.tile([128, 128], F32, tag="smallF")
            tp0 = smallF[:, 64:96]
            for k in range(4):
                nc.tensor.transpose(tp0[:, ts(k, 8)], h0sb[:, ts(k, 128)],
                                    i8f[:])
            nc.vector.tensor_copy(H2aT[:].rearrange("p a b -> p (a b)"), tp0[:])
            nc.vector.tensor_copy(H2bT[:].rearrange("p a b -> p (a b)"), tp0[:])

            c0ps = psS.tile([8, H], F32, tag="onetime")
            for k in range(4):
                nc.tensor.matmul(c0ps[:], sb_featT[:, k], sb_wic2[:, k],
                                 start=(k == 0), stop=False)
            nc.tensor.matmul(c0ps[:], ones_1x8[:], sb_bic2[:],
                             start=False, stop=True)
            nc.vector.tensor_copy(C2a[32:40], c0ps[:])
            nc.vector.tensor_copy(C2b[32:40], c0ps[:])

            # ---- one-time: enc_projT (+ bias folded in) ------------------
            sb_encT = sp.tile([128, 4, BP], BF16, tag="encT")
            dma3(sb_encT, encT)
            sb_wenc = sp.tile([128, 4, A], BF16, tag="wenc")
            dma3(sb_wenc, wenc)
            QS = BP // 4  # 392
            for k in range(4):          # a-chunk
                for q in range(4):      # bp quarter
                    eps = psS.tile([128, 512], F32, tag="onetime")
                    nc.tensor.matmul(eps[:, :QS], sb_biasad[:, ts(k, 128)],
                                     ones_row[:], start=True, stop=False)
                    for e in range(4):  # e-chunk
                        nc.tensor.matmul(
                            eps[:, :QS], sb_wenc[:, e, ts(k, 128)],
                            sb_encT[:, e, ts(q, QS)],
                            start=False, stop=(e == 3))
                    nc.scalar.copy(out=encproj[:, k, ts(q, QS)],
                                   in_=eps[:, :QS])

            # ---- one-time: we_part -> dram staging ----------------------
            sb_weT = sp.tile([128, 4, T * BL], BF16, tag="weT")
            dma3(sb_weT, weT)
            sb_wg0x = sp.tile([128, 4, G4], BF16, tag="wg0x")
            dma3(sb_wg0x, wg0x)
            sb_bg0 = sp.tile([1, G4], BF16, tag="bg0")
            nc.sync.dma_start(out=sb_bg0[:], in_=bg0)
            for m, rows in ((0, 128), (1, 32)):
                wsb = sp.tile([128, G4], BF16, tag="wepsb")
                for j in range(4):
                    wps = psS.tile([128, 512], F32, tag="onetime")
                    nc.tensor.matmul(wps[:rows, :],
                                     ones_1x128[:, :rows],
                                     sb_bg0[:, ts(j, 512)],
                                     start=True, stop=False)
                    for e in range(4):
                        nc.tensor.matmul(
                            wps[:rows, :],
                            sb_weT[:, e, m * 128:m * 128 + rows],
                            sb_wg0x[:, e, ts(j, 512)],
                            start=False, stop=(e == 3))
                    nc.vector.tensor_copy(wsb[:rows, ts(j, 512)],
                                          wps[:rows, :])
                nc.sync.dma_start(out=wepart[m * 128:m * 128 + rows],
                                  in_=wsb[:rows])

            sp.release()
            lp1 = tc.alloc_tile_pool(name="lp1", bufs=1)
            lp2 = tc.alloc_tile_pool(name="lp2", bufs=2)

            # ================= the recurrent steps ========================
            def gates0_early(t):
                """wepart inject + h_a-part of gates0 for step t (no ctx
                dependency): issued at the tail of step t-1 so the weight
                streams fill the PE during the pointwise + attention."""
                wet = lp2.tile([8, G4], BF16, tag="wet")
                nc.sync.dma_start(out=wet[:], in_=wepart[t * 8:(t + 1) * 8])
                g0 = psG.tile([128, 512], F32, tag="g")
                for j in range(4):
                    nc.tensor.matmul(g0[ts(j, 32), :], i8pad[:],
                                     wet[:, ts(j, 512)], start=True,
                                     stop=False, skip_group_check=True,
                                     tile_position=(0, 32 * j))
                for k in range(4):
                    for j in range(4):
                        nc.tensor.matmul(g0[32 * j:32 * j + 8, :],
                                         H2aT[:, k],
                                         sb_wg0h[:, k, ts(j, 512)],
                                         start=False, stop=False,
                                         skip_group_check=True,
                                         tile_position=(0, 32 * j))
                return g0

            g0_next = None
            for t in range(n_steps):
                g0 = g0_next if g0_next is not None else gates0_early(t)
                # --- decT = (0.5*W_dec).T @ H2b, computed transposed ------
                dps = smallF[:, 0:32]
                for c in range(4):
                    for k in range(4):
                        nc.tensor.matmul(dps[:, ts(c, 8)],
                                         sb_wdech[:, k, ts(c, 128)],
                                         H2bT[:, k],
                                         start=(k == 0), stop=(k == 3))
                decT = lp2.tile([128, 4, 8], BF16, tag="decT")
                nc.vector.tensor_copy(
                    decT[:].rearrange("p a b -> p (a b)"), dps[:])

                # --- gates1 early: bias inject + h_b(t-1) part; issued at
                # the step head so these weight streams fill the PE during
                # the attention add/tanh window.
                g1 = psG.tile([128, 512], F32, tag="g")
                for j in range(4):
                    nc.tensor.matmul(g1[ts(j, 32), :], onespad[:],
                                     sb_bg1[:, ts(j, 512)], start=True,
                                     stop=False, skip_group_check=True,
                                     tile_position=(0, 32 * j))
                for k in range(4):
                    for j in range(4):
                        nc.tensor.matmul(g1[32 * j:32 * j + 8, :],
                                         H2bT[:, k],
                                         sb_wg1b[:, k, ts(j, 512)],
                                         start=False, stop=False,
                                         skip_group_check=True,
                                         tile_position=(0, 32 * j))

                # --- s = tanh(enc_projT + dec_projT); scores = v^T s ------
                # scores land col-grouped: batch b at psum partition
                # 32*(b//2), free offset (b%2)*196 (one psum bank total).
                # Each k-chunk gets its own ssb tile so the tanh of chunk k
                # only depends on chunk k's add (precise region deps).
                ssbk = [lp1.tile([128, BP], BF16, tag=f"ssb{k}",
                                 name=f"ssbk{k}")
                        for k in range(4)]
                scps = psS.tile([128, 512], F32, tag="scps")
                for k in range(4):
                    eng = nc.vector if k < 2 else nc.gpsimd
                    eng.tensor_tensor(
                        ssbk[k][:].rearrange("p (b q) -> p b q", b=BL),
                        encproj[:, k].rearrange("p (b q) -> p b q", b=BL),
                        decT[:, k].unsqueeze(2).broadcast_to([128, BL, P]),
                        ALU.add)
                    nc.scalar.activation(out=ssbk[k][:], in_=ssbk[k][:],
                                         func=AF.Tanh)
                    for b in range(BL):
                        g, b2 = b // 2, b % 2
                        nc.tensor.matmul(
                            scps[32 * g:32 * g + 1, b2 * P:(b2 + 1) * P],
                            sb_vcol[:, k],
                            ssbk[k][:, b * P:(b + 1) * P],
                            start=(k == 0), stop=(k == 3),
                            skip_group_check=True,
                            tile_position=(0, 32 * g))

                # exp(scores): contiguous writes into b-major row [1, 1568]
                exp_row = lp2.tile([1, BP], BF16, tag="exprow")
                for g in range(4):
                    nc.scalar.activation(
                        out=exp_row[:, 2 * g * P:(2 * g + 2) * P],
                        in_=scps[32 * g:32 * g + 1, 0:2 * P],
                        func=AF.Exp)
                # per-batch exp sums (overlaps the alpha transposes)
                smro = lp2.tile([1, BL], F32, tag="smro")
                nc.vector.reduce_sum(
                    _sub(smro[:], [[1, BL], [0, 1]]),
                    _sub(exp_row[:], [[P, BL], [1, P]]),
                    axis=mybir.AxisListType.X)
                rinv_row = lp2.tile([1, 8], F32, tag="rinv")
                nc.vector.reciprocal(rinv_row[:], smro[:])
                # transpose the exp row into alpha columns on the PE (issued
                # before the selector build so the PE keeps streaming)
                # (psum cols 32:40 = rows 0:128, cols 40:48 = rows 128:196),
                # then two strided DVE copies build the aB block-columns.
                tpA = psT.tile([128, 64], BF16, tag="tpb", name="tpA")
                for b in range(BL):
                    nc.tensor.transpose(
                        tpA[:, 32 + 2 * b:33 + 2 * b],
                        exp_row[0:1, b * P:b * P + 128], i1b[:])
                    nc.tensor.transpose(
                        tpA[0:68, 48 + 2 * b:49 + 2 * b],
                        exp_row[0:1, b * P + 128:(b + 1) * P], i1b[:])
                nc.vector.tensor_copy(_sub(aB[:], [[17, 8]]),
                                      _sub(tpA[:], [[2, 8]], extra_offset=32))
                nc.vector.tensor_copy(
                    _sub(aB[0:68], [[17, 8]], extra_offset=8),
                    _sub(tpA[0:68], [[2, 8]], extra_offset=48))
                # context partials (4 col-groups), then selector-reduce to
                # transposed context [e, b] with the 1/sum scaling built in.
                cps4 = psS.tile([128, 512], F32, tag="cps4")
                for f in range(NF):
                    g = f % 4
                    if f < 4:
                        nc.tensor.matmul(cps4[ts(g, 32), :],
                                         aB[:, 8 * g:8 * g + 32],
                                         sb_encflat[:, f],
                                         start=True, stop=False,
                                         skip_group_check=True,
                                         tile_position=(0, 32 * g))
                    else:
                        nc.tensor.matmul(cps4[32 * g:32 * g + 8, :],
                                         aB[:, ts(f, 8)],
                                         sb_encflat[:, f],
                                         start=False, stop=(f >= NF - 4),
                                         skip_group_check=True,
                                         tile_position=(0, 32 * g))
                rbc = smallF[0:104, 104:112]
                nc.tensor.matmul(rbc, ones_1x104[:], rinv_row[:],
                                 start=True, stop=True,
                                 skip_group_check=True)
                sel = lp2.tile([104, BL], BF16, tag="sel")
                nc.vector.tensor_tensor(sel[:], sb_diag[:], rbc, ALU.mult)
                parts = lp1.tile([104, 512], BF16, tag="parts")
                nc.vector.tensor_copy(parts[:], cps4[0:104, :])
                ctps = smallF[:, 32:64]
                for c in range(4):
                    nc.tensor.matmul(ctps[:, ts(c, 8)],
                                     parts[:, ts(c, 128)], sel[:],
                                     start=True, stop=True,
                                     skip_group_check=True)
                ctxT = lp2.tile([128, 4, 8], BF16, tag="ctxT")
                nc.vector.tensor_copy(
                    ctxT[:].rearrange("p a b -> p (a b)"), ctps[:])

                # --- gates0 late: ctx part ---------------------------------
                for k in range(4):
                    for j in range(4):
                        nc.tensor.matmul(g0[32 * j:32 * j + 8, :],
                                         ctxT[:, k],
                                         sb_wg0c[:, k, ts(j, 512)],
                                         start=False, stop=(k == 3),
                                         skip_group_check=True,
                                         tile_position=(0, 32 * j))

                def lstm_pointwise(gps, C2):
                    """gates psum [128,512] (i@0,f@32,C~@64,o@96) -> h2.

                    All dual-SBUF-input vector ops use matching partition
                    bases: tgA holds (i@0-7, f@32-39), tgB holds
                    (C~@0-7, o@32-39); C2 state and s1/s2/tch live at
                    rows 32-39 of 40-partition tiles."""
                    tgA = lp2.tile([40, 512], F32, tag="pw_tgA")
                    nc.scalar.activation(out=tgA[:], in_=gps[0:40, :],
                                         func=AF.Tanh, scale=0.5)
                    tgB = lp2.tile([40, 512], F32, tag="pw_tgB")
                    nc.scalar.activation(out=tgB[:], in_=gps[64:104, :],
                                         func=AF.Tanh, scale=scvec2[:])
                    s1 = lp2.tile([40, H], F32, tag="pw_s1")
                    nc.vector.scalar_tensor_tensor(
                        out=s1[32:40], in0=tgA[32:40], scalar=1.0,
                        in1=C2[32:40], op0=ALU.add, op1=ALU.mult)
                    s2 = lp2.tile([40, H], F32, tag="pw_s2")
                    nc.vector.scalar_tensor_tensor(
                        out=s2[32:40], in0=tgA[0:8], scalar=1.0,
                        in1=tgB[0:8], op0=ALU.add, op1=ALU.mult)
                    nc.vector.scalar_tensor_tensor(
                        out=C2[32:40], in0=s1[32:40], scalar=0.5,
                        in1=s2[32:40], op0=ALU.mult, op1=ALU.add)
                    tch = lp2.tile([40, H], F32, tag="pw_tc")
                    nc.scalar.activation(out=tch[32:40], in_=C2[32:40],
                                         func=AF.Tanh, scale=0.5)
                    h2 = lp2.tile([8, H], BF16, tag="pw_h2")
                    nc.vector.scalar_tensor_tensor(
                        out=h2[:], in0=tgB[32:40], scalar=1.0,
                        in1=tch[32:40], op0=ALU.add, op1=ALU.mult)
                    return h2

                def transpose_h(h2, dest, hball_col=None):
                    tp = psT.tile([128, 64], BF16, tag="tpb", name="tp")
                    for k in range(4):
                        nc.tensor.transpose(tp[:, ts(k, 8)],
                                            h2[:, ts(k, 128)], i8b[:])
                    nc.vector.tensor_copy(
                        dest[:].rearrange("p a b -> p (a b)"), tp[:, 0:32])
                    if hball_col is not None:
                        nc.gpsimd.tensor_copy(
                            _sub(hball_sb[:], [[T * BL, 4], [1, 8]],
                                 extra_offset=hball_col),
                            dest[:].rearrange("p a b -> p (a b)"))

                h2a = lstm_pointwise(g0, C2a)
                transpose_h(h2a, H2aT)

                # --- gates1 late: h_a(t) part -----------------------------
                for k in range(4):
                    for j in range(4):
                        nc.tensor.matmul(g1[32 * j:32 * j + 8, :],
                                         H2aT[:, k],
                                         sb_wg1a[:, k, ts(j, 512)],
                                         start=False, stop=(k == 3),
                                         skip_group_check=True,
                                         tile_position=(0, 32 * j))

                g0_next = (gates0_early(t + 1) if t + 1 < n_steps else None)

                h2b = lstm_pointwise(g1, C2b)
                transpose_h(h2b, H2bT, hball_col=t * 8)

            for k in range(4):
                nc.sync.dma_start(out=hballT[k], in_=hball_sb[:, k])
            lp2.release()
            lp1.release()

    nc.compile()
    return nc


# --------------------------------------------------------------------------
# Phase B module: logitsT = (0.5*W_out).T @ H2b_all  (transposed layout)
# --------------------------------------------------------------------------

def build_phase_b(num_devices=NC):
    nc = bacc.Bacc("TRN2", num_devices=num_devices, debug=False)
    hbT = nc.dram_tensor("hbT", [4, 128, BT], BF16, kind="ExternalInput").ap()
    wout = nc.dram_tensor("wout", [4, 128, VSL], BF16,
                          kind="ExternalInput").ap()
    logitsT = nc.dram_tensor("logitsT", [VSL, BT], BF16,
                             kind="ExternalOutput").ap()

    vtiles = [(v, min(128, VSL - v)) for v in range(0, VSL, 128)]
    mtiles = [(m, min(512, BT - m)) for m in range(0, BT, 512)]
    VGRP = 6  # v-tiles per weight-prefetch group

    with tile.TileContext(nc) as tc:
        with (
            tc.tile_pool(name="w", bufs=1) as wp,
            tc.tile_pool(name="l", bufs=8) as lp,
            tc.tile_pool(name="ps", bufs=8, space="PSUM") as ps,
        ):
            sb_hbT = wp.tile([128, 4, BT], BF16, tag="hbT")
            for k in range(4):
                nc.sync.dma_start(out=sb_hbT[:, k], in_=hbT[k])
            # weight tile loaded in column groups so matmuls start early
            sb_wout = wp.tile([128, 4, VSL], BF16, tag="wout")
            for v0 in range(0, VSL, VGRP * 128):
                vw = min(VGRP * 128, VSL - v0)
                for k in range(4):
                    nc.sync.dma_start(
                        out=sb_wout[:, k, v0:v0 + vw],
                        in_=_sub(wout[k], [[1, vw]], extra_offset=v0))

            n = 0
            for (v0, vw) in vtiles:
                for (m0, mw) in mtiles:
                    pt = ps.tile([128, 512], F32, tag="acc")
                    for k in range(4):
                        nc.tensor.matmul(pt[:vw, :mw],
                                         sb_wout[:, k, v0:v0 + vw],
                                         sb_hbT[:, k, m0:m0 + mw],
                                         start=(k == 0), stop=(k == 3))
                    ot = lp.tile([128, 512], BF16, tag="out")
                    if n % 2 == 0:
                        nc.vector.tensor_copy(ot[:vw, :mw], pt[:vw, :mw])
                    else:
                        nc.scalar.copy(out=ot[:vw, :mw], in_=pt[:vw, :mw])
                    nc.sync.dma_start(out=logitsT[v0:v0 + vw, m0:m0 + mw],
                                      in_=ot[:vw, :mw])
                    n += 1
    nc.compile()
    return nc


# --------------------------------------------------------------------------
# Host-side preparation + driver
# --------------------------------------------------------------------------

def prep_phase_a_inputs(features, encoder_out, emb, W_enc, b_enc, W_dec, b_dec,
                        v_w, W_g0, b_g0, W_g1, b_g1, W_ih, b_ih, W_ic, b_ic,
                        captions):
    embeds = np.asarray(emb)[np.asarray(captions)[:, :T].astype(np.int64)]
    diag = np.zeros((104, BL), np.float32)
    for g in range(4):
        for b in range(BL):
            diag[32 * g + b, b] = 1.0
    shared = {
        "wih2": _bf16(2.0 * np.asarray(W_ih).reshape(4, 128, H)),
        "wic2": _bf16(2.0 * np.asarray(W_ic).reshape(4, 128, H)),
        "bih2": _bf16(2.0 * np.asarray(b_ih).reshape(1, H)),
        "bic2": _bf16(2.0 * np.asarray(b_ic).reshape(1, H)),
        "wenc": _bf16(np.asarray(W_enc).reshape(4, 128, A)),
        "biasad": _bf16((np.asarray(b_enc) + np.asarray(b_dec)).reshape(1, A)),
        "wdech": _bf16(0.5 * np.asarray(W_dec).reshape(4, 128, A)),
        "vcol": _bf16(np.asarray(v_w).reshape(4, 128, 1)),
        "wg0x": _bf16(_gorder(np.asarray(W_g0)[:E]).reshape(4, 128, G4)),
        "bg0": _bf16(_gorder(np.asarray(b_g0)).reshape(1, G4)),
        "wg0c": _bf16(_gorder(np.asarray(W_g0)[E:2 * E]).reshape(4, 128, G4)),
        "wg0h": _bf16(0.5 * _gorder(np.asarray(W_g0)[2 * E:])
                      .reshape(4, 128, G4)),
        "wg1a": _bf16(0.5 * _gorder(np.asarray(W_g1)[:H])
                      .reshape(4, 128, G4)),
        "wg1b": _bf16(0.5 * _gorder(np.asarray(W_g1)[H:])
                      .reshape(4, 128, G4)),
        "bg1": _bf16(_gorder(np.asarray(b_g1)).reshape(1, G4)),
        "diag01": _bf16(diag),
    }
    in_maps = []
    for c in range(NC):
        bs = slice(c * BL, (c + 1) * BL)
        enc = np.asarray(encoder_out)[bs]               # [8, 196, 512]
        encTn = enc.transpose(2, 0, 1).reshape(E, BL * P)
        encpad = np.zeros((BL, PPAD, E), np.float32)
        encpad[:, :P] = enc
        feat = np.asarray(features)[bs]
        we = embeds[bs]                                 # [8, T, E]
        m = dict(shared)
        m["encT"] = _bf16(encTn.reshape(4, 128, BL * P))
        m["encflat"] = _bf16(encpad.reshape(NF, 128, E))
        m["featT"] = _bf16(feat.T.reshape(4, 128, BL))
        m["weT"] = _bf16(we.transpose(2, 1, 0).reshape(4, 128, T * BL))
        in_maps.append(m)
    return in_maps


_CACHE = {}


def kernel(**inputs):
    inputs = {k: np.asarray(v) for k, v in inputs.items()}
    if "a" not in _CACHE:
        _CACHE["a"] = build_phase_a()
    if "b" not in _CACHE:
        _CACHE["b"] = build_phase_b()

    in_a = prep_phase_a_inputs(
        inputs["features"], inputs["encoder_out"], inputs["emb"],
        inputs["W_enc"], inputs["b_enc"], inputs["W_dec"], inputs["b_dec"],
        inputs["v_w"], inputs["W_g0"], inputs["b_g0"], inputs["W_g1"],
        inputs["b_g1"], inputs["W_ih"], inputs["b_ih"], inputs["W_ic"],
        inputs["b_ic"], inputs["captions"])
    ra = run_bass_kernel_spmd(_CACHE["a"], in_a, core_ids=list(range(NC)))

    # reassemble hb: column index b*T + t
    hbT_full = np.zeros((4, 128, BT), dtype=ml_dtypes.bfloat16)
    for c in range(NC):
        part = ra.results[c]["hballT"].reshape(4, 128, T, BL)
        for bl in range(BL):
            b = c * BL + bl
            hbT_full[:, :, b * T:(b + 1) * T] = part[:, :, :, bl]

    W_out = np.asarray(inputs["W_out"])
    b_out = np.asarray(inputs["b_out"])
    in_b = []
    for c in range(NC):
        vs = slice(c * VSL, (c + 1) * VSL)
        in_b.append({
            "hbT": hbT_full,
            "wout": _bf16(0.5 * W_out[:, vs].reshape(4, 128, VSL)),
        })
    rb = run_bass_kernel_spmd(_CACHE["b"], in_b, core_ids=list(range(NC)))
    logitsT = np.concatenate(
        [rb.results[c]["logitsT"].astype(np.float32) for c in range(NC)],
        axis=0)                                         # [V, B*T]
    logits = logitsT.T.reshape(B, T, V) + b_out[None, None, :]
    return logits.astype(np.float32)


# revision 30
# speedup vs baseline: 1.0277x; 1.0277x over previous
"""Trainium2 Bass kernel for nn_CaptionDecoder (attention LSTM caption decoder).

Strategy (8 NeuronCores):
  Phase A: data-parallel over batch (8 batches/core) for the sequential
           attention+2-layer-LSTM recurrence. Produces top-layer hidden
           states hb for all 20 steps (tiny: [512, 160] bf16 per core).
  Host:    gathers hb from the 8 cores (1.3 MB total), reassembles.
  Phase B: vocab-parallel logits projection: every core gets the full
           hb and a 3750-column slice of W_out; computes the logits
           TRANSPOSED ([3750, 1280]) with W_out as the matmul stationary
           operand (full 128-wide PE); the host concatenates along vocab
           and adds b_out.

Phase A layout tricks:
  - The two LSTM gate blocks [8, 2048] are computed with PE column-tiling:
    the four gate types (f, i, C~, o) each map to one 32-column group of
    the systolic array, so their weight streams execute concurrently
    (4x fewer serial N=512 passes).
  - dec_proj and the attention context are produced directly in
    TRANSPOSED form ([a/e, batch]) with the weights/encoder slices as
    stationary operands, eliminating 8 PE transposes per step.
  - Softmax plumbing: exp writes a p-major row so the alpha scatter into
    block-column form is 2 DMAs (was 16); exp-sums come back as a psum
    row whose reciprocal feeds a per-batch-scaled selector matrix that
    both reduces the context partials and applies 1/sum.
  - The four gate activations run as ONE 104-partition ACT instruction
    (groups live at partition bases 0/32/64/96) with a per-partition
    scale vector implementing the sigmoid-as-tanh trick.

Precision: bf16 matmuls with fp32 PSUM accumulation; fp32 state and
pointwise math. Sigmoid is computed as 0.5*(1+tanh(x/2)); the 0.5
factors are folded into pre-scaled weights by keeping doubled states
H2=2h, C2=2c on device.
"""

import numpy as np
import ml_dtypes

import concourse.bass as bass
import concourse.bacc as bacc
import concourse.mybir as mybir
import concourse.tile as tile
from concourse.bass import ts
from concourse.bass_utils import run_bass_kernel_spmd
from concourse.masks import make_identity

F32 = mybir.dt.float32
BF16 = mybir.dt.bfloat16
AF = mybir.ActivationFunctionType
ALU = mybir.AluOpType

B, TC, P, E, H, A, V = 64, 21, 196, 512, 512, 512, 30000
T = TC - 1            # 20 decode steps
NC = 8                # cores
BL = B // NC          # 8 batches per core
PPAD = 256            # padded attention positions per batch
NF = BL * PPAD // 128  # 16 position chunks for context matmul
BP = BL * P           # 1568 (b, p) columns per core
G4 = 4 * H            # 2048 stacked gates f,i,C,o
VSL = V // NC         # 3750 vocab columns per core
BT = B * T            # 1280 output rows


def _bf16(x):
    return np.ascontiguousarray(np.asarray(x), dtype=None).astype(ml_dtypes.bfloat16)


def _gorder(w):
    """Reorder stacked gate blocks (f,i,C,o) -> (i,f,C,o) on the last axis."""
    w = np.asarray(w)
    blocks = w.reshape(*w.shape[:-1], 4, H)
    return blocks[..., [1, 0, 2, 3], :].reshape(w.shape)


def _sub(ap, dims, extra_offset=0):
    """Custom free-dim access pattern on an AP, keeping its partition dim."""
    return bass.AP(ap.tensor, ap.offset + extra_offset,
                   [list(ap.ap[0])] + [list(d) for d in dims])


def _pbcast(ap, dims, nparts=128, extra_offset=0):
    """Partition-broadcast (stride 0) custom AP."""
    return bass.AP(ap.tensor, ap.offset + extra_offset,
                   [[0, nparts]] + [list(d) for d in dims])


# --------------------------------------------------------------------------
# Phase A module: the recurrence
# --------------------------------------------------------------------------

def build_phase_a(n_steps=T, num_devices=NC):
    nc = bacc.Bacc("TRN2", num_devices=num_devices, debug=False)

    def din(name, shape, dt=BF16):
        return nc.dram_tensor(name, shape, dt, kind="ExternalInput").ap()

    encT = din("encT", [4, 128, BP])          # encoder_out^T  [e-chk][e][(b,p)]
    encflat = din("encflat", [NF, 128, E])    # [(b,ppad) chk][row][e], 0-padded
    featT = din("featT", [4, 128, BL])
    wih2 = din("wih2", [4, 128, H])           # 2*W_ih
    wic2 = din("wic2", [4, 128, H])           # 2*W_ic
    bih2 = din("bih2", [1, H])
    bic2 = din("bic2", [1, H])
    wenc = din("wenc", [4, 128, A])
    biasad = din("biasad", [1, A])            # b_enc + b_dec
    wdech = din("wdech", [4, 128, A])         # 0.5*W_dec
    vcol = din("vcol", [4, 128, 1])
    weT = din("weT", [4, 128, T * BL])        # embeds^T, col = t*8+b
    wg0x = din("wg0x", [4, 128, G4])
    bg0 = din("bg0", [1, G4])
    wg0c = din("wg0c", [4, 128, G4])
    wg0h = din("wg0h", [4, 128, G4])          # 0.5*
    wg1a = din("wg1a", [4, 128, G4])          # 0.5*
    wg1b = din("wg1b", [4, 128, G4])          # 0.5*
    bg1 = din("bg1", [1, G4])
    diag01 = din("diag01", [104, BL])         # 1 at (32g+b, b)

    hballT = nc.dram_tensor("hballT", [4, 128, T * BL], BF16,
                            kind="ExternalOutput").ap()
    # staging for the hoisted word-embedding part of the gate0 pre-activation
    wepart = nc.dram_tensor("wepart", [T * BL, G4], BF16).ap()

    with tile.TileContext(nc) as tc:
        with (
            tc.tile_pool(name="persist", bufs=1) as pp,
            tc.tile_pool(name="psG", bufs=2, space="PSUM") as psG,
            tc.tile_pool(name="psS", bufs=1, space="PSUM") as psS,
            tc.tile_pool(name="psT", bufs=1, space="PSUM") as psT,
        ):
            def dma3(dst, src, n=4):  # dram [n,128,X] -> sbuf [128,n,X]
                for k in range(n):
                    nc.sync.dma_start(out=dst[:, k], in_=src[k])

            # ---- persistent weights / constants --------------------------
            sb_wdech = pp.tile([128, 4, A], BF16, tag="wdech")
            dma3(sb_wdech, wdech)
            sb_vcol = pp.tile([128, 4, 1], BF16, tag="vcol")
            dma3(sb_vcol, vcol)
            sb_biasad = pp.tile([1, A], BF16, tag="biasad")
            nc.sync.dma_start(out=sb_biasad[:], in_=biasad)
            sb_wg0c = pp.tile([128, 4, G4], BF16, tag="wg0c")
            dma3(sb_wg0c, wg0c)
            sb_wg0h = pp.tile([128, 4, G4], BF16, tag="wg0h")
            dma3(sb_wg0h, wg0h)
            sb_wg1a = pp.tile([128, 4, G4], BF16, tag="wg1a")
            dma3(sb_wg1a, wg1a)
            sb_wg1b = pp.tile([128, 4, G4], BF16, tag="wg1b")
            dma3(sb_wg1b, wg1b)
            sb_bg1 = pp.tile([1, G4], BF16, tag="bg1")
            nc.sync.dma_start(out=sb_bg1[:], in_=bg1)
            sb_encflat = pp.tile([128, NF, E], BF16, tag="encflat")
            dma3(sb_encflat, encflat, n=NF)
            sb_diag = pp.tile([104, BL], BF16, tag="diag01")
            nc.sync.dma_start(out=sb_diag[:], in_=diag01)

            i8f = pp.tile([8, 8], F32, tag="i8f")
            make_identity(nc, i8f[:])
            i8b = pp.tile([8, 8], BF16, tag="i8b")
            make_identity(nc, i8b[:])
            i1b = pp.tile([1, 1], BF16, tag="i1b")
            nc.vector.memset(i1b[:], 1.0)
            i8pad = pp.tile([8, 32], BF16, tag="i8pad")
            nc.vector.memset(i8pad[:], 0.0)
            make_identity(nc, i8pad[:, 0:8], nomemset=True)
            onespad = pp.tile([1, 32], BF16, tag="onespad")
            nc.vector.memset(onespad[:], 0.0)
            nc.vector.memset(onespad[:, 0:8], 1.0)
            ones_1x8 = pp.tile([1, 8], BF16, tag="o18")
            nc.vector.memset(ones_1x8[:], 1.0)
            ones_row = pp.tile([1, BP // 4], BF16, tag="orow")
            nc.vector.memset(ones_row[:], 1.0)
            ones_1x128 = pp.tile([1, 128], BF16, tag="o1128")
            nc.vector.memset(ones_1x128[:], 1.0)
            ones_col = pp.tile([128, 1], BF16, tag="ocol")
            nc.vector.memset(ones_col[:], 1.0)
            ones_1x104 = pp.tile([1, 104], F32, tag="o1104")
            nc.vector.memset(ones_1x104[:], 1.0)
            scvec2 = pp.tile([40, 1], F32, tag="scvec2")
            nc.vector.memset(scvec2[0:32], 1.0)
            nc.vector.memset(scvec2[32:40], 0.5)

            # state
            C2a = pp.tile([40, H], F32, tag="C2a")   # state in rows 32-39
            C2b = pp.tile([40, H], F32, tag="C2b")
            H2aT = pp.tile([128, 4, 8], BF16, tag="H2aT")
            H2bT = pp.tile([128, 4, 8], BF16, tag="H2bT")
            hball_sb = pp.tile([128, 4, T * BL], BF16, tag="hball")
            nc.vector.memset(hball_sb[:], 0.0)

            # alpha block-column tile: column 17*b of chunk-slice 2b holds
            # alpha[0:128] for batch b; column 17*b+8 holds alpha[128:196].
            aB = pp.tile([128, 128], BF16, tag="aB")
            nc.vector.memset(aB[:], 0.0)

            encproj = pp.tile([128, 4, BP], BF16, tag="encproj")

            # ---- one-time section (own pool, freed before the loop) ------
            sp = tc.alloc_tile_pool(name="stream", bufs=1)
            sb_featT = sp.tile([128, 4, BL], BF16, tag="featT")
            dma3(sb_featT, featT)
            sb_wih2 = sp.tile([128, 4, H], BF16, tag="wih2")
            dma3(sb_wih2, wih2)
            sb_wic2 = sp.tile([128, 4, H], BF16, tag="wic2")
            dma3(sb_wic2, wic2)
            sb_bih2 = sp.tile([1, H], BF16, tag="bih2")
            nc.sync.dma_start(out=sb_bih2[:], in_=bih2)
            sb_bic2 = sp.tile([1, H], BF16, tag="bic2")
            nc.sync.dma_start(out=sb_bic2[:], in_=bic2)

            h0ps = psS.tile([8, H], F32, tag="onetime0")
            for k in range(4):
                nc.tensor.matmul(h0ps[:], sb_featT[:, k], sb_wih2[:, k],
                                 start=(k == 0), stop=False)
            nc.tensor.matmul(h0ps[:], ones_1x8[:], sb_bih2[:],
                             start=False, stop=True)
            h0sb = sp.tile([8, H], F32, tag="h0sb")
            nc.vector.tensor_copy(h0sb[:], h0ps[:])
            smallF = psT.tile([128, 128], F32, tag="smallF")
            tp0 = smallF[:, 64:96]
            for k in range(4):
                nc.tensor.transpose(tp0[:, ts(k, 8)], h0sb[:, ts(k, 128)],
                                    i8f[:])
            nc.vector.tensor_copy(H2aT[:].rearrange("p a b -> p (a b)"), tp0[:])
            nc.vector.tensor_copy(H2bT[:].rearrange("p a b -> p (a b)"), tp0[:])

            c0ps = psS.tile([8, H], F32, tag="onetime1")
            for k in range(4):
                nc.tensor.matmul(c0ps[:], sb_featT[:, k], sb_wic2[:, k],
                                 start=(k == 0), stop=False)
            nc.tensor.matmul(c0ps[:], ones_1x8[:], sb_bic2[:],
                             start=False, stop=True)
            nc.vector.tensor_copy(C2a[32:40], c0ps[:])
            nc.vector.tensor_copy(C2b[32:40], c0ps[:])

            # ---- one-time: enc_projT (+ bias folded in) ------------------
            sb_encT = sp.tile([128, 4, BP], BF16, tag="encT")
            dma3(sb_encT, encT)
            sb_wenc = sp.tile([128, 4, A], BF16, tag="wenc")
            dma3(sb_wenc, wenc)
            QS = BP // 4  # 392
            for k in range(4):          # a-chunk
                for q in range(4):      # bp quarter
                    eps = psS.tile([128, 512], F32,
                                   tag=f"onetime{(k * 4 + q) % 2}",
                                   name="eps")
                    nc.tensor.matmul(eps[:, :QS], sb_biasad[:, ts(k, 128)],
                                     ones_row[:], start=True, stop=False)
                    for e in range(4):  # e-chunk
                        nc.tensor.matmul(
                            eps[:, :QS], sb_wenc[:, e, ts(k, 128)],
                            sb_encT[:, e, ts(q, QS)],
                            start=False, stop=(e == 3))
                    nc.scalar.copy(out=encproj[:, k, ts(q, QS)],
                                   in_=eps[:, :QS])

            # ---- one-time: we_part -> dram staging ----------------------
            sb_weT = sp.tile([128, 4, T * BL], BF16, tag="weT")
            dma3(sb_weT, weT)
            sb_wg0x = sp.tile([128, 4, G4], BF16, tag="wg0x")
            dma3(sb_wg0x, wg0x)
            sb_bg0 = sp.tile([1, G4], BF16, tag="bg0")
            nc.sync.dma_start(out=sb_bg0[:], in_=bg0)
            for m, rows in ((0, 128), (1, 32)):
                wsb = sp.tile([128, G4], BF16, tag="wepsb")
                for j in range(4):
                    wps = psS.tile([128, 512], F32, tag=f"onetime{j % 2}",
                                   name="wps")
                    nc.tensor.matmul(wps[:rows, :],
                                     ones_1x128[:, :rows],
                                     sb_bg0[:, ts(j, 512)],
                                     start=True, stop=False)
                    for e in range(4):
                        nc.tensor.matmul(
                            wps[:rows, :],
                            sb_weT[:, e, m * 128:m * 128 + rows],
                            sb_wg0x[:, e, ts(j, 512)],
                            start=False, stop=(e == 3))
                    nc.vector.tensor_copy(wsb[:rows, ts(j, 512)],
                                          wps[:rows, :])
                nc.sync.dma_start(out=wepart[m * 128:m * 128 + rows],
                                  in_=wsb[:rows])

            sp.release()
            lp1 = tc.alloc_tile_pool(name="lp1", bufs=1)
            lp2 = tc.alloc_tile_pool(name="lp2", bufs=2)

            # ================= the recurrent steps ========================
            def gates0_early(t):
                """wepart inject + h_a-part of gates0 for step t (no ctx
                dependency): issued at the tail of step t-1 so the weight
                streams fill the PE during the pointwise + attention."""
                wet = lp2.tile([8, G4], BF16, tag="wet")
                nc.sync.dma_start(out=wet[:], in_=wepart[t * 8:(t + 1) * 8])
                g0 = psG.tile([128, 512], F32, tag="g")
                for j in range(4):
                    nc.tensor.matmul(g0[ts(j, 32), :], i8pad[:],
                                     wet[:, ts(j, 512)], start=True,
                                     stop=False, skip_group_check=True,
                                     tile_position=(0, 32 * j))
                for k in range(4):
                    for j in range(4):
                        nc.tensor.matmul(g0[32 * j:32 * j + 8, :],
                                         H2aT[:, k],
                                         sb_wg0h[:, k, ts(j, 512)],
                                         start=False, stop=False,
                                         skip_group_check=True,
                                         tile_position=(0, 32 * j))
                return g0

            g0_next = None
            for t in range(n_steps):
                g0 = g0_next if g0_next is not None else gates0_early(t)
                # --- decT = (0.5*W_dec).T @ H2b, computed transposed ------
                dps = smallF[:, 0:32]
                for c in range(4):
                    for k in range(4):
                        nc.tensor.matmul(dps[:, ts(c, 8)],
                                         sb_wdech[:, k, ts(c, 128)],
                                         H2bT[:, k],
                                         start=(k == 0), stop=(k == 3))
                decT = lp2.tile([128, 4, 8], BF16, tag="decT")
                nc.vector.tensor_copy(
                    decT[:].rearrange("p a b -> p (a b)"), dps[:])

                # --- gates1 early: bias inject + h_b(t-1) part; issued at
                # the step head so these weight streams fill the PE during
                # the attention add/tanh window.
                g1 = psG.tile([128, 512], F32, tag="g")
                for j in range(4):
                    nc.tensor.matmul(g1[ts(j, 32), :], onespad[:],
                                     sb_bg1[:, ts(j, 512)], start=True,
                                     stop=False, skip_group_check=True,
                                     tile_position=(0, 32 * j))
                for k in range(4):
                    for j in range(4):
                        nc.tensor.matmul(g1[32 * j:32 * j + 8, :],
                                         H2bT[:, k],
                                         sb_wg1b[:, k, ts(j, 512)],
                                         start=False, stop=False,
                                         skip_group_check=True,
                                         tile_position=(0, 32 * j))

                # --- s = tanh(enc_projT + dec_projT); scores = v^T s ------
                # scores land col-grouped: batch b at psum partition
                # 32*(b//2), free offset (b%2)*196 (one psum bank total).
                # Each k-chunk gets its own ssb tile so the tanh of chunk k
                # only depends on chunk k's add (precise region deps).
                ssbk = [lp1.tile([128, BP], BF16, tag=f"ssb{k}",
                                 name=f"ssbk{k}")
                        for k in range(4)]
                scps = psS.tile([128, 512], F32, tag="scps")
                for k in range(4):
                    eng = nc.vector if k < 2 else nc.gpsimd
                    eng.tensor_tensor(
                        ssbk[k][:].rearrange("p (b q) -> p b q", b=BL),
                        encproj[:, k].rearrange("p (b q) -> p b q", b=BL),
                        decT[:, k].unsqueeze(2).broadcast_to([128, BL, P]),
                        ALU.add)
                    nc.scalar.activation(out=ssbk[k][:], in_=ssbk[k][:],
                                         func=AF.Tanh)
                    for b in range(BL):
                        g, b2 = b // 2, b % 2
                        nc.tensor.matmul(
                            scps[32 * g:32 * g + 1, b2 * P:(b2 + 1) * P],
                            sb_vcol[:, k],
                            ssbk[k][:, b * P:(b + 1) * P],
                            start=(k == 0), stop=(k == 3),
                            skip_group_check=True,
                            tile_position=(0, 32 * g))

                # exp(scores): contiguous writes into b-major row [1, 1568]
                exp_row = lp2.tile([1, BP], BF16, tag="exprow")
                for g in range(4):
                    nc.scalar.activation(
                        out=exp_row[:, 2 * g * P:(2 * g + 2) * P],
                        in_=scps[32 * g:32 * g + 1, 0:2 * P],
                        func=AF.Exp)
                # per-batch exp sums (overlaps the alpha transposes)
                smro = lp2.tile([1, BL], F32, tag="smro")
                nc.vector.reduce_sum(
                    _sub(smro[:], [[1, BL], [0, 1]]),
                    _sub(exp_row[:], [[P, BL], [1, P]]),
                    axis=mybir.AxisListType.X)
                rinv_row = lp2.tile([1, 8], F32, tag="rinv")
                nc.vector.reciprocal(rinv_row[:], smro[:])
                # transpose the exp row into alpha columns on the PE (issued
                # before the selector build so the PE keeps streaming)
                # (psum cols 32:40 = rows 0:128, cols 40:48 = rows 128:196),
                # then two strided DVE copies build the aB block-columns.
                tpA = psT.tile([128, 64], BF16, tag="tpb", name="tpA")
                for b in range(BL):
                    nc.tensor.transpose(
                        tpA[:, 32 + 2 * b:33 + 2 * b],
                        exp_row[0:1, b * P:b * P + 128], i1b[:])
                    nc.tensor.transpose(
                        tpA[0:68, 48 + 2 * b:49 + 2 * b],
                        exp_row[0:1, b * P + 128:(b + 1) * P], i1b[:])
                nc.vector.tensor_copy(_sub(aB[:], [[17, 8]]),
                                      _sub(tpA[:], [[2, 8]], extra_offset=32))
                nc.vector.tensor_copy(
                    _sub(aB[0:68], [[17, 8]], extra_offset=8),
                    _sub(tpA[0:68], [[2, 8]], extra_offset=48))
                # context partials (4 col-groups), then selector-reduce to
                # transposed context [e, b] with the 1/sum scaling built in.
                cps4 = psS.tile([128, 512], F32, tag="cps4")
                for f in range(NF):
                    g = f % 4
                    if f < 4:
                        nc.tensor.matmul(cps4[ts(g, 32), :],
                                         aB[:, 8 * g:8 * g + 32],
                                         sb_encflat[:, f],
                                         start=True, stop=False,
                                         skip_group_check=True,
                                         tile_position=(0, 32 * g))
                    else:
                        nc.tensor.matmul(cps4[32 * g:32 * g + 8, :],
                                         aB[:, ts(f, 8)],
                                         sb_encflat[:, f],
                                         start=False, stop=(f >= NF - 4),
                                         skip_group_check=True,
                                         tile_position=(0, 32 * g))
                rbc = smallF[0:104, 104:112]
                nc.tensor.matmul(rbc, ones_1x104[:], rinv_row[:],
                                 start=True, stop=True,
                                 skip_group_check=True)
                sel = lp2.tile([104, BL], BF16, tag="sel")
                nc.vector.tensor_tensor(sel[:], sb_diag[:], rbc, ALU.mult)
                parts = lp1.tile([104, 512], BF16, tag="parts")
                nc.vector.tensor_copy(parts[:], cps4[0:104, :])
                ctps = smallF[:, 32:64]
                for c in range(4):
                    nc.tensor.matmul(ctps[:, ts(c, 8)],
                                     parts[:, ts(c, 128)], sel[:],
                                     start=True, stop=True,
                                     skip_group_check=True)
                ctxT = lp2.tile([128, 4, 8], BF16, tag="ctxT")
                nc.vector.tensor_copy(
                    ctxT[:].rearrange("p a b -> p (a b)"), ctps[:])

                # --- gates0 late: ctx part ---------------------------------
                for k in range(4):
                    for j in range(4):
                        nc.tensor.matmul(g0[32 * j:32 * j + 8, :],
                                         ctxT[:, k],
                                         sb_wg0c[:, k, ts(j, 512)],
                                         start=False, stop=(k == 3),
                                         skip_group_check=True,
                                         tile_position=(0, 32 * j))

                def lstm_pointwise(gps, C2):
                    """gates psum [128,512] (i@0,f@32,C~@64,o@96) -> h2.

                    All dual-SBUF-input vector ops use matching partition
                    bases: tgA holds (i@0-7, f@32-39), tgB holds
                    (C~@0-7, o@32-39); C2 state and s1/s2/tch live at
                    rows 32-39 of 40-partition tiles."""
                    tgA = lp2.tile([40, 512], F32, tag="pw_tgA")
                    nc.scalar.activation(out=tgA[:], in_=gps[0:40, :],
                                         func=AF.Tanh, scale=0.5)
                    tgB = lp2.tile([40, 512], F32, tag="pw_tgB")
                    nc.scalar.activation(out=tgB[:], in_=gps[64:104, :],
                                         func=AF.Tanh, scale=scvec2[:])
                    s1 = lp2.tile([40, H], F32, tag="pw_s1")
                    nc.vector.scalar_tensor_tensor(
                        out=s1[32:40], in0=tgA[32:40], scalar=1.0,
                        in1=C2[32:40], op0=ALU.add, op1=ALU.mult)
                    s2 = lp2.tile([40, H], F32, tag="pw_s2")
                    nc.vector.scalar_tensor_tensor(
                        out=s2[32:40], in0=tgA[0:8], scalar=1.0,
                        in1=tgB[0:8], op0=ALU.add, op1=ALU.mult)
                    nc.vector.scalar_tensor_tensor(
                        out=C2[32:40], in0=s1[32:40], scalar=0.5,
                        in1=s2[32:40], op0=ALU.mult, op1=ALU.add)
                    tch = lp2.tile([40, H], F32, tag="pw_tc")
                    nc.scalar.activation(out=tch[32:40], in_=C2[32:40],
                                         func=AF.Tanh, scale=0.5)
                    h2 = lp2.tile([8, H], BF16, tag="pw_h2")
                    nc.vector.scalar_tensor_tensor(
                        out=h2[:], in0=tgB[32:40], scalar=1.0,
                        in1=tch[32:40], op0=ALU.add, op1=ALU.mult)
                    return h2

                def transpose_h(h2, dest, hball_col=None):
                    tp = psT.tile([128, 64], BF16, tag="tpb", name="tp")
                    for k in range(4):
                        nc.tensor.transpose(tp[:, ts(k, 8)],
                                            h2[:, ts(k, 128)], i8b[:])
                    nc.vector.tensor_copy(
                        dest[:].rearrange("p a b -> p (a b)"), tp[:, 0:32])
                    if hball_col is not None:
                        nc.gpsimd.tensor_copy(
                            _sub(hball_sb[:], [[T * BL, 4], [1, 8]],
                                 extra_offset=hball_col),
                            dest[:].rearrange("p a b -> p (a b)"))

                h2a = lstm_pointwise(g0, C2a)
                transpose_h(h2a, H2aT)

                # --- gates1 late: h_a(t) part -----------------------------
                for k in range(4):
                    for j in range(4):
                        nc.tensor.matmul(g1[32 * j:32 * j + 8, :],
                                         H2aT[:, k],
                                         sb_wg1a[:, k, ts(j, 512)],
                                         start=False, stop=(k == 3),
                                         skip_group_check=True,
                                         tile_position=(0, 32 * j))

                g0_next = (gates0_early(t + 1) if t + 1 < n_steps else None)

                h2b = lstm_pointwise(g1, C2b)
                transpose_h(h2b, H2bT, hball_col=t * 8)

            for k in range(4):
                nc.sync.dma_start(out=hballT[k], in_=hball_sb[:, k])
            lp2.release()
            lp1.release()

    nc.compile()
    return nc


# --------------------------------------------------------------------------
# Phase B module: logitsT = (0.5*W_out).T @ H2b_all  (transposed layout)
# --------------------------------------------------------------------------

def build_phase_b(num_devices=NC):
    nc = bacc.Bacc("TRN2", num_devices=num_devices, debug=False)
    hbT = nc.dram_tensor("hbT", [4, 128, BT], BF16, kind="ExternalInput").ap()
    wout = nc.dram_tensor("wout", [4, 128, VSL], BF16,
                          kind="ExternalInput").ap()
    logitsT = nc.dram_tensor("logitsT", [VSL, BT], BF16,
                             kind="ExternalOutput").ap()

    vtiles = [(v, min(128, VSL - v)) for v in range(0, VSL, 128)]
    mtiles = [(m, min(512, BT - m)) for m in range(0, BT, 512)]
    VGRP = 6  # v-tiles per weight-prefetch group

    with tile.TileContext(nc) as tc:
        with (
            tc.tile_pool(name="w", bufs=1) as wp,
            tc.tile_pool(name="l", bufs=8) as lp,
            tc.tile_pool(name="ps", bufs=8, space="PSUM") as ps,
        ):
            sb_hbT = wp.tile([128, 4, BT], BF16, tag="hbT")
            for k in range(4):
                nc.sync.dma_start(out=sb_hbT[:, k], in_=hbT[k])
            # weight tile loaded in column groups so matmuls start early
            sb_wout = wp.tile([128, 4, VSL], BF16, tag="wout")
            for v0 in range(0, VSL, VGRP * 128):
                vw = min(VGRP * 128, VSL - v0)
                for k in range(4):
                    nc.sync.dma_start(
                        out=sb_wout[:, k, v0:v0 + vw],
                        in_=_sub(wout[k], [[1, vw]], extra_offset=v0))

            n = 0
            for (v0, vw) in vtiles:
                for (m0, mw) in mtiles:
                    pt = ps.tile([128, 512], F32, tag="acc")
                    for k in range(4):
                        nc.tensor.matmul(pt[:vw, :mw],
                                         sb_wout[:, k, v0:v0 + vw],
                                         sb_hbT[:, k, m0:m0 + mw],
                                         start=(k == 0), stop=(k == 3))
                    ot = lp.tile([128, 512], BF16, tag="out")
                    if n % 2 == 0:
                        nc.vector.tensor_copy(ot[:vw, :mw], pt[:vw, :mw])
                    else:
                        nc.scalar.copy(out=ot[:vw, :mw], in_=pt[:vw, :mw])
                    nc.sync.dma_start(out=logitsT[v0:v0 + vw, m0:m0 + mw],
                                      in_=ot[:vw, :mw])
                    n += 1
    nc.compile()
    return nc


# --------------------------------------------------------------------------
# Host-side preparation + driver
# --------------------------------------------------------------------------

def prep_phase_a_inputs(features, encoder_out, emb, W_enc, b_enc, W_dec, b_dec,
                        v_w, W_g0, b_g0, W_g1, b_g1, W_ih, b_ih, W_ic, b_ic,
                        captions):
    embeds = np.asarray(emb)[np.asarray(captions)[:, :T].astype(np.int64)]
    diag = np.zeros((104, BL), np.float32)
    for g in range(4):
        for b in range(BL):
            diag[32 * g + b, b] = 1.0
    shared = {
        "wih2": _bf16(2.0 * np.asarray(W_ih).reshape(4, 128, H)),
        "wic2": _bf16(2.0 * np.asarray(W_ic).reshape(4, 128, H)),
        "bih2": _bf16(2.0 * np.asarray(b_ih).reshape(1, H)),
        "bic2": _bf16(2.0 * np.asarray(b_ic).reshape(1, H)),
        "wenc": _bf16(np.asarray(W_enc).reshape(4, 128, A)),
        "biasad": _bf16((np.asarray(b_enc) + np.asarray(b_dec)).reshape(1, A)),
        "wdech": _bf16(0.5 * np.asarray(W_dec).reshape(4, 128, A)),
        "vcol": _bf16(np.asarray(v_w).reshape(4, 128, 1)),
        "wg0x": _bf16(_gorder(np.asarray(W_g0)[:E]).reshape(4, 128, G4)),
        "bg0": _bf16(_gorder(np.asarray(b_g0)).reshape(1, G4)),
        "wg0c": _bf16(_gorder(np.asarray(W_g0)[E:2 * E]).reshape(4, 128, G4)),
        "wg0h": _bf16(0.5 * _gorder(np.asarray(W_g0)[2 * E:])
                      .reshape(4, 128, G4)),
        "wg1a": _bf16(0.5 * _gorder(np.asarray(W_g1)[:H])
                      .reshape(4, 128, G4)),
        "wg1b": _bf16(0.5 * _gorder(np.asarray(W_g1)[H:])
                      .reshape(4, 128, G4)),
        "bg1": _bf16(_gorder(np.asarray(b_g1)).reshape(1, G4)),
        "diag01": _bf16(diag),
    }
    in_maps = []
    for c in range(NC):
        bs = slice(c * BL, (c + 1) * BL)
        enc = np.asarray(encoder_out)[bs]               # [8, 196, 512]
        encTn = enc.transpose(2, 0, 1).reshape(E, BL * P)
        encpad = np.zeros((BL, PPAD, E), np.float32)
        encpad[:, :P] = enc
        feat = np.asarray(features)[bs]
        we = embeds[bs]                                 # [8, T, E]
        m = dict(shared)
        m["encT"] = _bf16(encTn.reshape(4, 128, BL * P))
        m["encflat"] = _bf16(encpad.reshape(NF, 128, E))
        m["featT"] = _bf16(feat.T.reshape(4, 128, BL))
        m["weT"] = _bf16(we.transpose(2, 1, 0).reshape(4, 128, T * BL))
        in_maps.append(m)
    return in_maps


_CACHE = {}


def kernel(**inputs):
    inputs = {k: np.asarray(v) for k, v in inputs.items()}
    if "a" not in _CACHE:
        _CACHE["a"] = build_phase_a()
    if "b" not in _CACHE:
        _CACHE["b"] = build_phase_b()

    in_a = prep_phase_a_inputs(
        inputs["features"], inputs["encoder_out"], inputs["emb"],
        inputs["W_enc"], inputs["b_enc"], inputs["W_dec"], inputs["b_dec"],
        inputs["v_w"], inputs["W_g0"], inputs["b_g0"], inputs["W_g1"],
        inputs["b_g1"], inputs["W_ih"], inputs["b_ih"], inputs["W_ic"],
        inputs["b_ic"], inputs["captions"])
    ra = run_bass_kernel_spmd(_CACHE["a"], in_a, core_ids=list(range(NC)))

    # reassemble hb: column index b*T + t
    hbT_full = np.zeros((4, 128, BT), dtype=ml_dtypes.bfloat16)
    for c in range(NC):
        part = ra.results[c]["hballT"].reshape(4, 128, T, BL)
        for bl in range(BL):
            b = c * BL + bl
            hbT_full[:, :, b * T:(b + 1) * T] = part[:, :, :, bl]

    W_out = np.asarray(inputs["W_out"])
    b_out = np.asarray(inputs["b_out"])
    in_b = []
    for c in range(NC):
        vs = slice(c * VSL, (c + 1) * VSL)
        in_b.append({
            "hbT": hbT_full,
            "wout": _bf16(0.5 * W_out[:, vs].reshape(4, 128, VSL)),
        })
    rb = run_bass_kernel_spmd(_CACHE["b"], in_b, core_ids=list(range(NC)))
    logitsT = np.concatenate(
        [rb.results[c]["logitsT"].astype(np.float32) for c in range(NC)],
        axis=0)                                         # [V, B*T]
    logits = logitsT.T.reshape(B, T, V) + b_out[None, None, :]
    return logits.astype(np.float32)
